# revision 19
# baseline (speedup 1.0000x reference)
"""Trainium2 Bass kernel for a pre-norm transformer decoder layer.

Sharding: 8 cores = 4 batches x 2 sequence-stripe halves.
Core c handles batch b=c//2 and the 1024 queries q with (q mod 512)//256 == c%2
(alternating 256-wide stripes -> causal-load-balanced and the per-core
program is identical across cores; only input data differs).

On-chip dataflow is feature-major ("transposed"): activations live as
[D, tokens] so every matmul contracts over the partition dim with zero
on-chip transposes.  The host pre-transposes x and all weights and
transposes the output back.

Matmul operands are bf16 (full PE rate); accumulation fp32 in PSUM.
LayerNorm statistics use ones-matmuls over partition chunks; mean/rstd rows
are broadcast across partitions with K=1 outer-product matmuls (float32r).
Causal softmax skips max-subtraction (scores are O(1) here) and uses a
multiplicative 0/1 bf16 mask after exp; the denominator rides along as an
extra ones-row in the attn@V accumulation.
"""

import numpy as np
import ml_dtypes

import concourse.bass as bass
import concourse.tile as tile
import concourse.mybir as mybir
from concourse.bass_utils import run_bass_kernel_spmd

FP32 = mybir.dt.float32
F32R = mybir.dt.float32r
BF16 = mybir.dt.bfloat16
AOP = mybir.AluOpType
ACT = mybir.ActivationFunctionType
EPS = 1e-5


def _split_drain_waits(nc, max_waits=1):
    """walrus in this container rejects >max_waits sync waits per
    instruction; split extras onto preceding single-wait NoOps on the same
    engine (engine streams execute in program order, so the waits still
    gate the real instruction)."""
    for f in nc.m.functions:
        for bb in f.blocks:
            insts = list(bb.instructions)
            out, changed = [], False
            for inst in insts:
                si = inst.sync_info
                if si is not None and len(si.on_wait) > max_waits:
                    waits = list(si.on_wait)
                    for j, w in enumerate(waits[:-max_waits]):
                        out.append(mybir.InstNoOp(
                            name=f"{inst.name}_sw{j}", ins=[], outs=[],
                            engine=inst.engine,
                            sync_info=mybir.SyncInfo(on_wait=[w], on_update=[])))
                    inst.sync_info = mybir.SyncInfo(
                        on_wait=waits[-max_waits:], on_update=list(si.on_update))
                    changed = True
                out.append(inst)
            if changed:
                bb.instructions = out


def build_decoder_nc(S=2048, D=1024, F=4096, apply_ln_affine=False, debug=False, surgery=True):
    """Build the single-core Bass program (shapes per core)."""
    DC = D // 128           # feature chunks
    FC = F // 128           # ffn feature chunks
    OWN = S // 2            # tokens owned by this core
    P = S // 512            # query pairs (tq=256 each)
    NKT = S // 128          # k-tiles over full sequence
    TS = 512                # token tile
    NT = S // TS
    TSO = min(TS, OWN)
    NTO = OWN // TSO
    scale_q = 1.0 / float(np.sqrt(D))

    nc = bass.Bass()

    # ---- DRAM I/O ----
    xTbf = nc.dram_tensor("xTbf", [D, S], BF16, kind="ExternalInput")
    xTobf = nc.dram_tensor("xTobf", [D, OWN], BF16, kind="ExternalInput")
    xTown = nc.dram_tensor("xTown", [D, OWN], FP32, kind="ExternalInput")
    mask01 = nc.dram_tensor("mask01", [512, 256], BF16, kind="ExternalInput")
    wqT = nc.dram_tensor("wqT", [D, D], BF16, kind="ExternalInput")
    wkT = nc.dram_tensor("wkT", [D, D], BF16, kind="ExternalInput")
    wvT = nc.dram_tensor("wvT", [D, D], BF16, kind="ExternalInput")
    woT = nc.dram_tensor("woT", [D, D], BF16, kind="ExternalInput")
    fc1T = nc.dram_tensor("fc1T", [D, F], BF16, kind="ExternalInput")
    fc2T = nc.dram_tensor("fc2T", [F, D], BF16, kind="ExternalInput")
    fc1b = nc.dram_tensor("fc1b", [F], FP32, kind="ExternalInput")
    fc2b = nc.dram_tensor("fc2b", [D], FP32, kind="ExternalInput")
    lnp = None
    if apply_ln_affine:
        lnp = nc.dram_tensor("lnp", [4, D], FP32, kind="ExternalInput")
    outT = nc.dram_tensor("outT", [D, OWN], FP32, kind="ExternalOutput")
    y1d = nc.dram_tensor("y1d", [D, OWN], FP32,
                         kind="ExternalOutput" if debug else "Internal")
    if debug:
        dbg_q = nc.dram_tensor("dbg_q", [D, OWN], FP32, kind="ExternalOutput")
        dbg_k = nc.dram_tensor("dbg_k", [D, S], FP32, kind="ExternalOutput")
        dbg_v = nc.dram_tensor("dbg_v", [S, D], FP32, kind="ExternalOutput")
        dbg_pt = nc.dram_tensor("dbg_pt", [4, 128, 256], FP32,
                                kind="ExternalOutput")
        dbg_den = nc.dram_tensor("dbg_den", [1, 256], FP32,
                                 kind="ExternalOutput")
        dbg_ctxn = nc.dram_tensor("dbg_ctxn", [D, 256], FP32,
                                  kind="ExternalOutput")
        dbg_densb = nc.dram_tensor("dbg_densb", [128, 256], FP32,
                                   kind="ExternalOutput")
        dbg_xn1 = nc.dram_tensor("dbg_xn1", [D, S], FP32,
                                 kind="ExternalOutput")

    xTbf_r = xTbf.rearrange("(c p) s -> p c s", p=128)
    xTobf_r = xTobf.rearrange("(c p) s -> p c s", p=128)
    xTown_r = xTown.rearrange("(c p) s -> p c s", p=128)
    mask_r = mask01.rearrange("(j p) t -> p j t", p=128)
    wqT_r = wqT.rearrange("(c p) e -> p c e", p=128)
    wkT_r = wkT.rearrange("(c p) e -> p c e", p=128)
    wvT_r = wvT.rearrange("(c p) e -> p c e", p=128)
    woT_r = woT.rearrange("(c p) e -> p c e", p=128)
    fc1T_r = fc1T.rearrange("(c p) f -> p c f", p=128)
    fc2T_r = fc2T.rearrange("(c p) d -> p c d", p=128)
    fc1b_r = fc1b.rearrange("(c p) -> p c", p=128)
    fc2b_r = fc2b.rearrange("(c p) -> p c", p=128)
    outT_r = outT.rearrange("(c p) s -> p c s", p=128)
    y1d_r = y1d.rearrange("(c p) s -> p c s", p=128)

    with tile.TileContext(nc) as tc:
        with (
            tc.tile_pool(name="consts", bufs=1) as consts,
            tc.tile_pool(name="work", bufs=2) as work,
        ):
            # constants
            ones_col = consts.tile([128, 1], BF16, tag="ones_col")
            nc.vector.memset(ones_col, 1.0)
            ones_row = consts.tile([1, 128], F32R, tag="ones_row")
            ones_row_f = consts.tile([1, 128], FP32, tag="ones_row_f")
            nc.vector.memset(ones_row_f, 1.0)
            nc.vector.tensor_copy(ones_row, ones_row_f)
            ones_colf = consts.tile([128, 1], FP32, tag="ones_colf")
            nc.vector.memset(ones_colf, 1.0)
            eps_t = consts.tile([1, 1], FP32, tag="eps")
            nc.vector.memset(eps_t, EPS)
            fc1b_t = consts.tile([128, FC], FP32, tag="fc1b")
            nc.sync.dma_start(fc1b_t, fc1b_r)
            fc2b_t = consts.tile([128, DC], FP32, tag="fc2b")
            nc.sync.dma_start(fc2b_t, fc2b_r)
            maskt = consts.tile([128, 4, 256], BF16, tag="mask")
            nc.sync.dma_start(maskt, mask_r)
            lnp_t = None
            if apply_ln_affine:
                lnp_t = consts.tile([128, 4, DC], FP32, tag="lnp")
                nc.sync.dma_start(
                    lnp_t, lnp.rearrange("g (c p) -> p g c", p=128))

            def ln_stats(src_tile, ntiles, ts, rows_pool, ln_ps):
                """src_tile: [128, DC, ntiles*ts] bf16 (or fp32) ->
                (mu, rstd) rows [1, ntiles*ts] fp32."""
                is_bf = src_tile.dtype == BF16
                mu = rows_pool.tile([1, ntiles * ts], F32R, tag="mu")
                rstd = rows_pool.tile([1, ntiles * ts], F32R, tag="rs")
                musq = rows_pool.tile([1, ntiles * ts], F32R, tag="msq")
                for ti in range(ntiles):
                    sl = slice(ti * ts, (ti + 1) * ts)
                    ps_sx = ln_ps.tile([1, ts], FP32, tag="ps_sx")
                    ps_sq = ln_ps.tile([1, ts], FP32, tag="ps_sq")
                    for dc in range(DC):
                        src = src_tile[:, dc, sl]
                        sq = work.tile([128, ts], src_tile.dtype, tag="lnsq")
                        nc.scalar.square(sq, src)
                        if is_bf:
                            nc.tensor.matmul(ps_sx, ones_col, src,
                                             start=(dc == 0),
                                             stop=(dc == DC - 1))
                            nc.tensor.matmul(ps_sq, ones_col, sq,
                                             start=(dc == 0),
                                             stop=(dc == DC - 1))
                        else:
                            nc.tensor.matmul(
                                ps_sx, ones_colf.bitcast(F32R),
                                src.bitcast(F32R), start=(dc == 0),
                                stop=(dc == DC - 1))
                            nc.tensor.matmul(
                                ps_sq, ones_colf.bitcast(F32R),
                                sq.bitcast(F32R), start=(dc == 0),
                                stop=(dc == DC - 1))
                    nc.scalar.activation(mu[0:1, sl], ps_sx, ACT.Copy,
                                         scale=1.0 / D)
                    nc.scalar.activation(rstd[0:1, sl], ps_sq, ACT.Copy,
                                         scale=1.0 / D)
                # var = E[x^2] - mu^2 ; rstd = 1/sqrt(var+eps)
                nc.vector.tensor_mul(musq, mu, mu)
                nc.vector.tensor_sub(rstd, rstd, musq)
                nc.scalar.activation(rstd, rstd, ACT.Sqrt, bias=eps_t)
                with nc.allow_low_precision(reason="rstd row in f32r for bcast matmul"):
                    nc.vector.reciprocal(rstd, rstd)
                return mu, rstd

            def ln_apply(src_tile, dst_tile, mu, rstd, ntiles, ts,
                         g_idx, b_idx, bc_ps):
                """dst[:,dc,t] = (src - mu[t]) * rstd[t] (* g + b).
                dst may alias src."""
                for ti in range(ntiles):
                    sl = slice(ti * ts, (ti + 1) * ts)
                    mb = bc_ps.tile([128, ts], FP32, tag="mb")
                    rb = bc_ps.tile([128, ts], FP32, tag="rb")
                    nc.tensor.matmul(mb, ones_row,
                                     mu[0:1, sl])
                    nc.tensor.matmul(rb, ones_row,
                                     rstd[0:1, sl])
                    for dc in range(DC):
                        t1 = work.tile([128, ts], FP32, tag="lnt1")
                        nc.vector.tensor_sub(t1, src_tile[:, dc, sl], mb)
                        if apply_ln_affine:
                            nc.vector.scalar_tensor_tensor(
                                dst_tile[:, dc, sl], t1,
                                lnp_t[:, g_idx, dc:dc + 1], rb,
                                AOP.mult, AOP.mult)
                            nc.vector.tensor_scalar_add(
                                dst_tile[:, dc, sl], dst_tile[:, dc, sl],
                                lnp_t[:, b_idx, dc:dc + 1])
                        else:
                            nc.vector.tensor_mul(dst_tile[:, dc, sl], t1, rb)

            with tc.tile_pool(name="L1", bufs=1) as L1:
                KT = L1.tile([128, DC, S], BF16, tag="KT")
                VT = L1.tile([128, NKT, D], BF16, tag="VT")
                QT = L1.tile([128, DC, OWN], BF16, tag="QT")
                WO = L1.tile([128, DC, D], BF16, tag="WO")
                for dc in range(DC):
                    nc.sync.dma_start(WO[:, dc, :], woT_r[:, dc, :])

                with (
                    tc.tile_pool(name="ln_psA", bufs=1, space="PSUM") as lnpsA,
                    tc.tile_pool(name="bc_psA", bufs=1, space="PSUM") as bcpsA,
                    tc.tile_pool(name="proj_ps", bufs=3, space="PSUM") as pps,
                ):
                    with tc.tile_pool(name="xn1p", bufs=1) as xn1p:
                        xn1 = xn1p.tile([128, DC, S], BF16, tag="xn1")
                        for dc in range(DC):
                            nc.sync.dma_start(xn1[:, dc, :], xTbf_r[:, dc, :])
                        with tc.tile_pool(name="rows1", bufs=1) as rows1:
                            mu1, rs1 = ln_stats(xn1, NT, TS, rows1, lnpsA)
                            ln_apply(xn1, xn1, mu1, rs1, NT, TS, 0, 1, bcpsA)

                        with tc.tile_pool(name="wpool", bufs=1) as wp:
                            WQ = wp.tile([128, DC, D], BF16, tag="w")
                            for dc in range(DC):
                                nc.sync.dma_start(WQ[:, dc, :], wqT_r[:, dc, :])
                            with tc.tile_pool(name="xn1op", bufs=1) as xn1op:
                                xn1o = xn1op.tile([128, DC, OWN], BF16,
                                                  tag="xn1o")
                                for dc in range(DC):
                                    nc.sync.dma_start(xn1o[:, dc, :],
                                                      xTobf_r[:, dc, :])
                                with tc.tile_pool(name="rows1o", bufs=1) as r1o:
                                    mu1o, rs1o = ln_stats(xn1o, NTO, TSO,
                                                          r1o, lnpsA)
                                    ln_apply(xn1o, xn1o, mu1o, rs1o, NTO, TSO,
                                             0, 1, bcpsA)
                                # Q projection (scaled 1/sqrt(D))
                                for ec in range(DC):
                                    for tj in range(NTO):
                                        sl = slice(tj * TSO, (tj + 1) * TSO)
                                        ps = pps.tile([128, TSO], FP32,
                                                      tag="pps")
                                        for dc in range(DC):
                                            nc.tensor.matmul(
                                                ps,
                                                WQ[:, dc,
                                                   ec * 128:(ec + 1) * 128],
                                                xn1o[:, dc, sl],
                                                start=(dc == 0),
                                                stop=(dc == DC - 1))
                                        nc.scalar.activation(
                                            QT[:, ec, sl], ps, ACT.Copy,
                                            scale=scale_q)
                            # K projection
                            WK = wp.tile([128, DC, D], BF16, tag="w")
                            for dc in range(DC):
                                nc.sync.dma_start(WK[:, dc, :], wkT_r[:, dc, :])
                            for ec in range(DC):
                                for ti in range(NT):
                                    sl = slice(ti * TS, (ti + 1) * TS)
                                    ps = pps.tile([128, TS], FP32, tag="pps")
                                    for dc in range(DC):
                                        nc.tensor.matmul(
                                            ps,
                                            WK[:, dc, ec * 128:(ec + 1) * 128],
                                            xn1[:, dc, sl],
                                            start=(dc == 0),
                                            stop=(dc == DC - 1))
                                    nc.scalar.activation(KT[:, ec, sl], ps,
                                                         ACT.Copy)
                            # V projection (token-major)
                            WV = wp.tile([128, DC, D], BF16, tag="w")
                            for dc in range(DC):
                                nc.sync.dma_start(WV[:, dc, :], wvT_r[:, dc, :])
                            EH = min(512, D)
                            for tcn in range(NKT):
                                for eh in range(D // EH):
                                    esl = slice(eh * EH, (eh + 1) * EH)
                                    ps = pps.tile([128, EH], FP32, tag="pps")
                                    for dc in range(DC):
                                        nc.tensor.matmul(
                                            ps,
                                            xn1[:, dc,
                                                tcn * 128:(tcn + 1) * 128],
                                            WV[:, dc, esl],
                                            start=(dc == 0),
                                            stop=(dc == DC - 1))
                                    nc.scalar.activation(VT[:, tcn, esl], ps,
                                                         ACT.Copy)

                if debug:
                    with tc.tile_pool(name="dbgp", bufs=2) as dbp:
                        for dc in range(DC):
                            dt_ = dbp.tile([128, S], FP32, tag="d1")
                            nc.vector.tensor_copy(dt_, KT[:, dc, :])
                            nc.sync.dma_start(
                                dbg_k.rearrange("(c p) s -> p c s", p=128)[:, dc, :], dt_)
                            dt2 = dbp.tile([128, OWN], FP32, tag="d2")
                            nc.vector.tensor_copy(dt2, QT[:, dc, :])
                            nc.sync.dma_start(
                                dbg_q.rearrange("(c p) s -> p c s", p=128)[:, dc, :], dt2)
                        for tcn in range(NKT):
                            dt4 = dbp.tile([128, D], FP32, tag="d3")
                            nc.vector.tensor_copy(dt4, VT[:, tcn, :])
                            nc.sync.dma_start(
                                dbg_v.rearrange("(c p) d -> p c d", p=128)[:, tcn, :], dt4)

                # ---- attention ----
                with (
                    tc.tile_pool(name="s_ps", bufs=3, space="PSUM") as sps,
                    tc.tile_pool(name="av_ps", bufs=3, space="PSUM") as avp,
                    tc.tile_pool(name="den_ps", bufs=1, space="PSUM") as dps,
                    tc.tile_pool(name="ptp", bufs=16) as ptp,
                    tc.tile_pool(name="pep", bufs=3) as pep,
                    tc.tile_pool(name="attw", bufs=2) as attw,
                    tc.tile_pool(name="denr", bufs=2) as denr,
                ):
                    for p in range(P):
                        qsl = slice(p * 256, (p + 1) * 256)
                        nkt = (p + 1) * 4
                        dn = dps.tile([1, 256], FP32, tag="dn")
                        pts = []
                        for kt in range(nkt):
                            s_ps = sps.tile([128, 256], FP32, tag="s")
                            for ec in range(DC):
                                nc.tensor.matmul(
                                    s_ps, KT[:, ec, kt * 128:(kt + 1) * 128],
                                    QT[:, ec, qsl],
                                    start=(ec == 0), stop=(ec == DC - 1))
                            pt = ptp.tile([128, 256], BF16, tag="pt")
                            if kt // 4 == p:  # diagonal superblock
                                pe = pep.tile([128, 256], BF16, tag="pe")
                                nc.scalar.activation(pe, s_ps, ACT.Exp)
                                nc.vector.tensor_mul(pt, pe,
                                                     maskt[:, kt % 4, :])
                            else:
                                nc.scalar.activation(pt, s_ps, ACT.Exp)
                            if debug and p == 0:
                                dpt = attw.tile([128, 256], FP32, tag="dpt")
                                nc.vector.tensor_copy(dpt, pt)
                                nc.sync.dma_start(dbg_pt[kt, :, :], dpt)
                            # denominator accumulates in its own bank
                            nc.tensor.matmul(dn, ones_col, pt,
                                             start=(kt == 0),
                                             stop=(kt == nkt - 1))
                            pts.append(pt)
                        den = denr.tile([1, 256], F32R, tag="den")
                        nc.vector.tensor_copy(den, dn)
                        with nc.allow_low_precision(reason="softmax denom"):
                            nc.vector.reciprocal(den, den)
                        den_b = dps.tile([128, 256], FP32, tag="denb")
                        nc.tensor.matmul(den_b, ones_row, den)
                        if debug and p == 0:
                            dden = attw.tile([1, 256], FP32, tag="dden")
                            nc.vector.tensor_copy(dden, den)
                            nc.sync.dma_start(dbg_den[:, :], dden)
                        den_sb = attw.tile([128, 256], FP32, tag="densb")
                        nc.scalar.activation(den_sb, den_b, ACT.Copy)
                        # attn@V: one bank-exclusive accumulation group per dc
                        ctxn = attw.tile([128, DC, 256], BF16, tag="ctxn")
                        for dc in range(DC):
                            cps = avp.tile([128, 256], FP32, tag="av")
                            for kt in range(nkt):
                                nc.tensor.matmul(
                                    cps, VT[:, kt, dc * 128:(dc + 1) * 128],
                                    pts[kt], start=(kt == 0),
                                    stop=(kt == nkt - 1))
                            nc.vector.tensor_mul(ctxn[:, dc, :], cps, den_sb)
                        if debug and p == 0:
                            ddsb = attw.tile([128, 256], FP32, tag="ddsb")
                            nc.vector.tensor_copy(ddsb, den_sb)
                            nc.sync.dma_start(dbg_densb[:, :], ddsb)
                            for dc in range(DC):
                                dcx = attw.tile([128, 256], FP32, tag="dcx")
                                nc.vector.tensor_copy(dcx, ctxn[:, dc, :])
                                nc.sync.dma_start(
                                    dbg_ctxn.rearrange("(c p) t -> p c t", p=128)[:, dc, :], dcx)
                        # O-projection + residual -> y1 (DRAM)
                        xo_t = attw.tile([128, DC, 256], FP32, tag="xo_t")
                        y1_t = attw.tile([128, DC, 256], FP32, tag="y1_t")
                        for ec in range(DC):
                            nc.sync.dma_start(xo_t[:, ec, :],
                                              xTown_r[:, ec, qsl])
                            ops_t = avp.tile([128, 256], FP32, tag="av")
                            for dc in range(DC):
                                nc.tensor.matmul(
                                    ops_t,
                                    WO[:, dc, ec * 128:(ec + 1) * 128],
                                    ctxn[:, dc, :],
                                    start=(dc == 0), stop=(dc == DC - 1))
                            nc.vector.tensor_add(y1_t[:, ec, :], ops_t,
                                                 xo_t[:, ec, :])
                            nc.sync.dma_start(y1d_r[:, ec, qsl], y1_t[:, ec, :])

            # ---- LN2 + FFN ----
            with tc.tile_pool(name="hpool", bufs=1) as hp:
                h = hp.tile([128, FC, OWN], BF16, tag="h")
                with (
                    tc.tile_pool(name="bigw", bufs=2) as bw,
                    tc.tile_pool(name="ln_psB", bufs=1, space="PSUM") as lnpsB,
                    tc.tile_pool(name="bc_psB", bufs=1, space="PSUM") as bcpsB,
                    tc.tile_pool(name="ffn_ps", bufs=3, space="PSUM") as fps,
                    tc.tile_pool(name="outp", bufs=3) as otp,
                ):
                    with tc.tile_pool(name="xn2p", bufs=1) as xn2p:
                        xn2 = xn2p.tile([128, DC, OWN], BF16, tag="xn2")
                        with (
                            tc.tile_pool(name="y1s", bufs=3) as y1s,
                            tc.tile_pool(name="rows2", bufs=1) as rows2,
                        ):
                            # LN2: stats pass streaming y1 from DRAM
                            mu2 = rows2.tile([1, OWN], F32R, tag="mu")
                            rs2 = rows2.tile([1, OWN], F32R, tag="rs")
                            msq2 = rows2.tile([1, OWN], F32R, tag="msq")
                            for tj in range(NTO):
                                sl = slice(tj * TSO, (tj + 1) * TSO)
                                ps_sx = lnpsB.tile([1, TSO], FP32, tag="ps_sx")
                                ps_sq = lnpsB.tile([1, TSO], FP32, tag="ps_sq")
                                for dc in range(DC):
                                    yt = y1s.tile([128, TSO], FP32, tag="yt")
                                    nc.sync.dma_start(yt, y1d_r[:, dc, sl])
                                    ybf = work.tile([128, TSO], BF16, tag="ybf")
                                    nc.scalar.activation(ybf, yt, ACT.Copy)
                                    sq = work.tile([128, TSO], BF16, tag="lnsq2")
                                    nc.scalar.square(sq, ybf)
                                    nc.tensor.matmul(
                                        ps_sx, ones_col, ybf, start=(dc == 0),
                                        stop=(dc == DC - 1))
                                    nc.tensor.matmul(
                                        ps_sq, ones_col, sq, start=(dc == 0),
                                        stop=(dc == DC - 1))
                                nc.scalar.activation(mu2[0:1, sl], ps_sx,
                                                     ACT.Copy, scale=1.0 / D)
                                nc.scalar.activation(rs2[0:1, sl], ps_sq,
                                                     ACT.Copy, scale=1.0 / D)
                            nc.vector.tensor_mul(msq2, mu2, mu2)
                            nc.vector.tensor_sub(rs2, rs2, msq2)
                            nc.scalar.activation(rs2, rs2, ACT.Sqrt, bias=eps_t)
                            with nc.allow_low_precision(reason="rstd row f32r"):
                                nc.vector.reciprocal(rs2, rs2)
                            # apply pass (stream y1 again)
                            for tj in range(NTO):
                                sl = slice(tj * TSO, (tj + 1) * TSO)
                                mb = bcpsB.tile([128, TSO], FP32, tag="mb")
                                rb = bcpsB.tile([128, TSO], FP32, tag="rb")
                                nc.tensor.matmul(mb, ones_row,
                                                 mu2[0:1, sl])
                                nc.tensor.matmul(rb, ones_row,
                                                 rs2[0:1, sl])
                                for dc in range(DC):
                                    yt = y1s.tile([128, TSO], FP32, tag="yt")
                                    nc.sync.dma_start(yt, y1d_r[:, dc, sl])
                                    t1 = work.tile([128, TSO], FP32, tag="lnt1")
                                    nc.vector.tensor_sub(t1, yt, mb)
                                    if apply_ln_affine:
                                        nc.vector.scalar_tensor_tensor(
                                            xn2[:, dc, sl], t1,
                                            lnp_t[:, 2, dc:dc + 1], rb,
                                            AOP.mult, AOP.mult)
                                        nc.vector.tensor_scalar_add(
                                            xn2[:, dc, sl], xn2[:, dc, sl],
                                            lnp_t[:, 3, dc:dc + 1])
                                    else:
                                        nc.vector.tensor_mul(xn2[:, dc, sl],
                                                             t1, rb)
                        # fc1 + relu (stream fc1T in halves)
                        FH = FC // 2
                        for half in range(2):
                            w1 = bw.tile([128, DC, FH * 128], BF16, tag="bigw")
                            for dc in range(DC):
                                nc.sync.dma_start(
                                    w1[:, dc, :],
                                    fc1T_r[:, dc,
                                           half * FH * 128:(half + 1) * FH * 128])
                            for fi in range(FH):
                                fc = half * FH + fi
                                for tj in range(NTO):
                                    sl = slice(tj * TSO, (tj + 1) * TSO)
                                    ps = fps.tile([128, TSO], FP32, tag="fps")
                                    for dc in range(DC):
                                        nc.tensor.matmul(
                                            ps,
                                            w1[:, dc, fi * 128:(fi + 1) * 128],
                                            xn2[:, dc, sl],
                                            start=(dc == 0),
                                            stop=(dc == DC - 1))
                                    nc.scalar.activation(
                                        h[:, fc, sl], ps, ACT.Relu,
                                        bias=fc1b_t[:, fc:fc + 1])
                    # fc2 in two halves; half0 -> partial (+bias), then
                    # half1 adds partial + streamed y1 -> out
                    with tc.tile_pool(name="ffacc", bufs=1) as fap:
                        FH2 = FC // 2
                        ffa = fap.tile([128, DC, OWN], FP32, tag="ffa")
                        for half in range(2):
                            w2 = bw.tile([128, FH2, D], BF16, tag="bigw")
                            for fi in range(FH2):
                                nc.sync.dma_start(
                                    w2[:, fi, :], fc2T_r[:, half * FH2 + fi, :])
                            for dc in range(DC):
                                for tj in range(NTO):
                                    sl = slice(tj * TSO, (tj + 1) * TSO)
                                    ps = fps.tile([128, TSO], FP32, tag="fps")
                                    for fi in range(FH2):
                                        fc = half * FH2 + fi
                                        nc.tensor.matmul(
                                            ps, w2[:, fi, dc * 128:(dc + 1) * 128],
                                            h[:, fc, sl],
                                            start=(fi == 0),
                                            stop=(fi == FH2 - 1))
                                    if half == 0:
                                        # ffa = psum + fc2_b
                                        nc.vector.tensor_scalar_add(
                                            ffa[:, dc, sl], ps,
                                            fc2b_t[:, dc:dc + 1])
                                    else:
                                        yt = otp.tile([128, TSO], FP32,
                                                      tag="yt2")
                                        nc.sync.dma_start(yt, y1d_r[:, dc, sl])
                                        ot = otp.tile([128, TSO], FP32,
                                                      tag="ot")
                                        nc.vector.tensor_add(ot, ps,
                                                             ffa[:, dc, sl])
                                        nc.vector.tensor_add(ot, ot, yt)
                                        nc.sync.dma_start(outT_r[:, dc, sl], ot)

    if surgery:
        _split_drain_waits(nc)
    return nc


# ---------------- host side ----------------

_NC_CACHE = {}


def _get_nc(S, D, F, apply_ln_affine):
    key = (S, D, F, apply_ln_affine)
    if key not in _NC_CACHE:
        _NC_CACHE[key] = build_decoder_nc(S, D, F, apply_ln_affine)
    return _NC_CACHE[key]


def make_in_maps(x, W_q, W_k, W_v, W_o, fc1_w, fc1_b, fc2_w, fc2_b,
                 ln1_g, ln1_b, ln2_g, ln2_b, apply_ln_affine):
    B, S, D = x.shape
    bf = ml_dtypes.bfloat16
    shared = {
        "wqT": np.ascontiguousarray(W_q.T).astype(bf),
        "wkT": np.ascontiguousarray(W_k.T).astype(bf),
        "wvT": np.ascontiguousarray(W_v.T).astype(bf),
        "woT": np.ascontiguousarray(W_o.T).astype(bf),
        "fc1T": np.ascontiguousarray(fc1_w.T).astype(bf),
        "fc2T": np.ascontiguousarray(fc2_w.T).astype(bf),
        "fc1b": np.ascontiguousarray(fc1_b, dtype=np.float32),
        "fc2b": np.ascontiguousarray(fc2_b, dtype=np.float32),
    }
    if apply_ln_affine:
        shared["lnp"] = np.ascontiguousarray(
            np.stack([ln1_g, ln1_b, ln2_g, ln2_b]), dtype=np.float32)
    in_maps, stripes = [], []
    for c in range(2 * B):
        b, hh = c // 2, c % 2
        stripe = (np.arange(S) % 512) // 256 == hh
        stripes.append((b, stripe))
        xTb = np.ascontiguousarray(x[b].T, dtype=np.float32)
        m = np.zeros((512, 256), dtype=bf)
        tk = np.arange(512)[:, None]
        j = np.arange(256)[None, :]
        m[tk <= j + 256 * hh] = 1.0
        xTo = np.ascontiguousarray(xTb[:, stripe])
        in_maps.append(dict(shared,
                            xTbf=xTb.astype(bf),
                            xTobf=xTo.astype(bf),
                            xTown=xTo,
                            mask01=m))
    return in_maps, stripes


def run_decoder(x, W_q, W_k, W_v, W_o, fc1_w, fc1_b, fc2_w, fc2_b,
                ln1_g, ln1_b, ln2_g, ln2_b, trace=False):
    x = np.asarray(x, dtype=np.float32)
    B, S, D = x.shape
    F = fc1_w.shape[0]
    apply_ln_affine = not (
        np.all(np.asarray(ln1_g) == 1.0) and np.all(np.asarray(ln1_b) == 0.0)
        and np.all(np.asarray(ln2_g) == 1.0) and np.all(np.asarray(ln2_b) == 0.0))
    nc = _get_nc(S, D, F, apply_ln_affine)
    in_maps, stripes = make_in_maps(
        x, np.asarray(W_q), np.asarray(W_k), np.asarray(W_v),
        np.asarray(W_o), np.asarray(fc1_w), np.asarray(fc1_b),
        np.asarray(fc2_w), np.asarray(fc2_b), np.asarray(ln1_g),
        np.asarray(ln1_b), np.asarray(ln2_g), np.asarray(ln2_b),
        apply_ln_affine)
    res = run_bass_kernel_spmd(nc, in_maps, core_ids=list(range(2 * B)),
                               trace=trace)
    out = np.empty((B, S, D), dtype=np.float32)
    for c in range(2 * B):
        b, stripe = stripes[c]
        out[b, stripe, :] = res.results[c]["outT"].T
    return out, res


def kernel(**inputs):
    out, _ = run_decoder(**inputs)
    return out


def time_decoder(iters=20, **inputs):
    """Estimate per-execution HW time by timing repeated PJRT executions of
    the compiled NEFF with device-resident inputs (no donation, no re-upload).
    Returns (best_ns, mean_ns)."""
    import time
    import jax
    from jax.sharding import Mesh, PartitionSpec, NamedSharding
    from jax.experimental.shard_map import shard_map
    from concourse import bass2jax
    from concourse import mybir as mb

    x = np.asarray(inputs["x"], dtype=np.float32)
    B, S, D = x.shape
    F = np.asarray(inputs["fc1_w"]).shape[0]
    apply_ln_affine = not (
        np.all(np.asarray(inputs["ln1_g"]) == 1.0)
        and np.all(np.asarray(inputs["ln1_b"]) == 0.0)
        and np.all(np.asarray(inputs["ln2_g"]) == 1.0)
        and np.all(np.asarray(inputs["ln2_b"]) == 0.0))
    nc = _get_nc(S, D, F, apply_ln_affine)
    in_maps, _ = make_in_maps(
        x, *[np.asarray(inputs[k]) for k in
             ("W_q", "W_k", "W_v", "W_o", "fc1_w", "fc1_b", "fc2_w", "fc2_b",
              "ln1_g", "ln1_b", "ln2_g", "ln2_b")], apply_ln_affine)
    n_cores = 2 * B

    bass2jax.install_neuronx_cc_hook()
    partition_name = (nc.partition_id_tensor.name
                      if nc.partition_id_tensor else None)
    in_names, out_names, out_avals, zero_outs = [], [], [], []
    for alloc in nc.m.functions[0].allocations:
        if not isinstance(alloc, mybir.MemoryLocationSet):
            continue
        name = alloc.memorylocations[0].name
        if alloc.kind == "ExternalInput":
            if name != partition_name:
                in_names.append(name)
        elif alloc.kind == "ExternalOutput":
            shape = tuple(alloc.tensor_shape)
            dtype = mybir.dt.np(alloc.dtype)
            out_names.append(name)
            out_avals.append(jax.core.ShapedArray(shape, dtype))
            zero_outs.append(np.zeros(shape, dtype))
    n_params = len(in_names)
    in_names.extend(out_names)
    if partition_name is not None:
        in_names.append(partition_name)

    def _body(*args):
        operands = list(args)
        if partition_name is not None:
            operands.append(bass2jax.partition_id_tensor())
        return tuple(bass2jax._bass_exec_p.bind(
            *operands, out_avals=tuple(out_avals), in_names=tuple(in_names),
            out_names=tuple(out_names), lowering_input_output_aliases=(),
            sim_require_finite=True, sim_require_nnan=True, nc=nc))

    devices = jax.devices()[:n_cores]
    mesh = Mesh(np.asarray(devices), ("core",))
    in_specs = (PartitionSpec("core"),) * (n_params + len(out_names))
    out_specs = (PartitionSpec("core"),) * len(out_names)
    fn = jax.jit(shard_map(_body, mesh=mesh, in_specs=in_specs,
                           out_specs=out_specs, check_rep=False),
                 keep_unused=True)
    sh = NamedSharding(mesh, PartitionSpec("core"))
    args = []
    for i in range(n_params):
        cat = np.concatenate([np.asarray(in_maps[c][in_names[i]])
                              for c in range(n_cores)], axis=0)
        args.append(jax.device_put(cat, sh))
    for z in zero_outs:
        cat = np.zeros((n_cores * z.shape[0], *z.shape[1:]), z.dtype)
        args.append(jax.device_put(cat, sh))

    outs = fn(*args)
    jax.block_until_ready(outs)
    times = []
    for _ in range(iters):
        t0 = time.perf_counter()
        outs = fn(*args)
        jax.block_until_ready(outs)
        times.append((time.perf_counter() - t0) * 1e9)
    return min(times), float(np.mean(times))


# revision 24
# speedup vs baseline: 93.8414x; 93.8414x over previous
"""Trainium2 Bass kernel for a pre-norm transformer decoder layer.

Sharding: 8 cores = 4 batches x 2 sequence-stripe halves.
Core c handles batch b=c//2 and the 1024 queries q with (q mod 512)//256 == c%2
(alternating 256-wide stripes -> causal-load-balanced and the per-core
program is identical across cores; only input data differs).

On-chip dataflow is feature-major ("transposed"): activations live as
[D, tokens]; every matmul contracts over the partition dim with zero on-chip
transposes (host pre-transposes x and all weights, output transposed back).
Matmul operands are bf16 (full PE rate), fp32 PSUM accumulation.
LayerNorm statistics use ones-matmuls; mean/rstd rows broadcast across
partitions via K=1 outer-product matmuls (float32r).  LN1 is pipelined
per token-tile with the K/V projections; LN2 is folded into the attention
pair loop (stats+apply on the in-SBUF y1 tiles).  Causal softmax skips
max-subtraction (scores are O(1) here) and applies a multiplicative 0/1
bf16 mask after exp; the softmax denominator accumulates in its own PSUM
bank alongside attn@V.  PSUM accumulation groups are strictly
bank-exclusive (a group's first matmul zeroes its whole bank).
"""

import numpy as np
import ml_dtypes

import concourse.bass as bass
import concourse.tile as tile
import concourse.mybir as mybir
from concourse.bass_utils import run_bass_kernel_spmd

FP32 = mybir.dt.float32
F32R = mybir.dt.float32r
BF16 = mybir.dt.bfloat16
AOP = mybir.AluOpType
ACT = mybir.ActivationFunctionType
EPS = 1e-5


def _split_drain_waits(nc, max_waits=1):
    """walrus here rejects >max_waits sync waits per instruction; split
    extras onto preceding single-wait NoOps on the same engine."""
    for f in nc.m.functions:
        for bb in f.blocks:
            insts = list(bb.instructions)
            out, changed = [], False
            for inst in insts:
                si = inst.sync_info
                if si is not None and len(si.on_wait) > max_waits:
                    waits = list(si.on_wait)
                    for j, w in enumerate(waits[:-max_waits]):
                        out.append(mybir.InstNoOp(
                            name=f"{inst.name}_sw{j}", ins=[], outs=[],
                            engine=inst.engine,
                            sync_info=mybir.SyncInfo(on_wait=[w],
                                                     on_update=[])))
                    inst.sync_info = mybir.SyncInfo(
                        on_wait=waits[-max_waits:],
                        on_update=list(si.on_update))
                    changed = True
                out.append(inst)
            if changed:
                bb.instructions = out


def build_decoder_nc(S=2048, D=1024, F=4096, apply_ln_affine=False,
                     debug=False, surgery=True, repeat=1):
    """Single-core Bass program (per-core shapes)."""
    DC = D // 128
    FC = F // 128
    OWN = S // 2
    P = S // 512
    NKT = S // 128
    TS = 512
    NT = S // TS
    TSO = min(TS, OWN)
    NTO = OWN // TSO
    scale_q = 1.0 / float(np.sqrt(D))

    nc = bass.Bass()

    xTbf = nc.dram_tensor("xTbf", [D, S], BF16, kind="ExternalInput")
    xTobf = nc.dram_tensor("xTobf", [D, OWN], BF16, kind="ExternalInput")
    xTown = nc.dram_tensor("xTown", [D, OWN], FP32, kind="ExternalInput")
    mask01 = nc.dram_tensor("mask01", [512, 256], BF16, kind="ExternalInput")
    wqT = nc.dram_tensor("wqT", [D, D], BF16, kind="ExternalInput")
    wkT = nc.dram_tensor("wkT", [D, D], BF16, kind="ExternalInput")
    wvT = nc.dram_tensor("wvT", [D, D], BF16, kind="ExternalInput")
    woT = nc.dram_tensor("woT", [D, D], BF16, kind="ExternalInput")
    fc1T = nc.dram_tensor("fc1T", [D, F], BF16, kind="ExternalInput")
    fc2T = nc.dram_tensor("fc2T", [F, D], BF16, kind="ExternalInput")
    fc1b = nc.dram_tensor("fc1b", [F], FP32, kind="ExternalInput")
    fc2b = nc.dram_tensor("fc2b", [D], FP32, kind="ExternalInput")
    lnp = None
    if apply_ln_affine:
        lnp = nc.dram_tensor("lnp", [4, D], FP32, kind="ExternalInput")
    outT = nc.dram_tensor("outT", [D, OWN], FP32, kind="ExternalOutput")
    y1d = nc.dram_tensor("y1d", [D, OWN], FP32, kind="Internal")

    xTbf_r = xTbf.rearrange("(c p) s -> p c s", p=128)
    xTobf_r = xTobf.rearrange("(c p) s -> p c s", p=128)
    xTown_r = xTown.rearrange("(c p) s -> p c s", p=128)
    mask_r = mask01.rearrange("(j p) t -> p j t", p=128)
    wqT_r = wqT.rearrange("(c p) e -> p c e", p=128)
    wkT_r = wkT.rearrange("(c p) e -> p c e", p=128)
    wvT_r = wvT.rearrange("(c p) e -> p c e", p=128)
    woT_r = woT.rearrange("(c p) e -> p c e", p=128)
    fc1T_r = fc1T.rearrange("(c p) f -> p c f", p=128)
    fc2T_r = fc2T.rearrange("(c p) d -> p c d", p=128)
    fc1b_r = fc1b.rearrange("(c p) -> p c", p=128)
    fc2b_r = fc2b.rearrange("(c p) -> p c", p=128)
    outT_r = outT.rearrange("(c p) s -> p c s", p=128)
    y1d_r = y1d.rearrange("(c p) s -> p c s", p=128)

    with tile.TileContext(nc) as tc:
        with (
            tc.tile_pool(name="consts", bufs=1) as consts,
            tc.tile_pool(name="work", bufs=2) as work,
        ):
            ones_col = consts.tile([128, 1], BF16, tag="ones_col")
            nc.vector.memset(ones_col, 1.0)
            ones_row = consts.tile([1, 128], F32R, tag="ones_row")
            ones_row_f = consts.tile([1, 128], FP32, tag="ones_row_f")
            nc.vector.memset(ones_row_f, 1.0)
            nc.vector.tensor_copy(ones_row, ones_row_f)
            eps_t = consts.tile([1, 1], FP32, tag="eps")
            nc.vector.memset(eps_t, EPS)
            fc1b_t = consts.tile([128, FC], FP32, tag="fc1b")
            nc.sync.dma_start(fc1b_t, fc1b_r)
            fc2b_t = consts.tile([128, DC], FP32, tag="fc2b")
            nc.sync.dma_start(fc2b_t, fc2b_r)
            maskt = consts.tile([128, 4, 256], BF16, tag="mask")
            nc.sync.dma_start(maskt, mask_r)
            lnp_t = None
            if apply_ln_affine:
                lnp_t = consts.tile([128, 4, DC], FP32, tag="lnp")
                nc.sync.dma_start(
                    lnp_t, lnp.rearrange("g (c p) -> p g c", p=128))

            def ln_tile(src3, dst3, ti, ts, ln_ps, bc_ps, g_idx, b_idx):
                """LayerNorm stats+apply for one bf16 token tile (in-place
                allowed)."""
                sl = slice(ti * ts, (ti + 1) * ts)
                ps_sx = ln_ps.tile([1, ts], FP32, tag="ps_sx")
                ps_sq = ln_ps.tile([1, ts], FP32, tag="ps_sq")
                for dc in range(DC):
                    src = src3[:, dc, sl]
                    sq = work.tile([128, ts], BF16, tag="lnsq")
                    nc.scalar.square(sq, src)
                    nc.tensor.matmul(ps_sx, ones_col, src,
                                     start=(dc == 0), stop=(dc == DC - 1))
                    nc.tensor.matmul(ps_sq, ones_col, sq,
                                     start=(dc == 0), stop=(dc == DC - 1))
                mu = work.tile([1, ts], F32R, tag="r_mu")
                rs = work.tile([1, ts], F32R, tag="r_rs")
                msq = work.tile([1, ts], F32R, tag="r_msq")
                nc.scalar.activation(mu, ps_sx, ACT.Copy, scale=1.0 / D)
                nc.scalar.activation(rs, ps_sq, ACT.Copy, scale=1.0 / D)
                nc.vector.tensor_mul(msq, mu, mu)
                nc.vector.tensor_sub(rs, rs, msq)
                nc.scalar.activation(rs, rs, ACT.Sqrt, bias=eps_t)
                with nc.allow_low_precision(reason="rstd row f32r"):
                    nc.vector.reciprocal(rs, rs)
                mb = bc_ps.tile([128, ts], FP32, tag="bc")
                nc.tensor.matmul(mb, ones_row, mu)
                mb_sb = work.tile([128, ts], FP32, tag="mb_sb")
                nc.scalar.activation(mb_sb, mb, ACT.Copy)
                rb = bc_ps.tile([128, ts], FP32, tag="bc")
                nc.tensor.matmul(rb, ones_row, rs)
                rb_sb = work.tile([128, ts], FP32, tag="rb_sb")
                nc.scalar.activation(rb_sb, rb, ACT.Copy)
                for dc in range(DC):
                    t1 = work.tile([128, ts], FP32, tag="lnt1")
                    nc.vector.tensor_sub(t1, src3[:, dc, sl], mb_sb)
                    if apply_ln_affine:
                        nc.vector.scalar_tensor_tensor(
                            dst3[:, dc, sl], t1,
                            lnp_t[:, g_idx, dc:dc + 1], rb_sb,
                            AOP.mult, AOP.mult)
                        nc.vector.tensor_scalar_add(
                            dst3[:, dc, sl], dst3[:, dc, sl],
                            lnp_t[:, b_idx, dc:dc + 1])
                    else:
                        nc.vector.tensor_mul(dst3[:, dc, sl], t1, rb_sb)

            for _rep in range(repeat):
              with tc.tile_pool(name="xn2p", bufs=1) as xn2p:
                xn2 = xn2p.tile([128, DC, OWN], BF16, tag="xn2")
                with tc.tile_pool(name="L1", bufs=1) as L1:
                    KT = L1.tile([128, DC, S], BF16, tag="KT")
                    VT = L1.tile([128, NKT, D], BF16, tag="VT")
                    QT = L1.tile([128, DC, OWN], BF16, tag="QT")

                    with (
                        tc.tile_pool(name="lnpsA", bufs=1,
                                     space="PSUM") as lnpsA,
                        tc.tile_pool(name="bcpsA", bufs=2,
                                     space="PSUM") as bcpsA,
                        tc.tile_pool(name="projps", bufs=3,
                                     space="PSUM") as pps,
                        tc.tile_pool(name="wpool", bufs=2) as wp,
                    ):
                        WK = wp.tile([128, DC, D], BF16, tag="w")
                        for dc in range(DC):
                            nc.sync.dma_start(WK[:, dc, :], wkT_r[:, dc, :])
                        WV = wp.tile([128, DC, D], BF16, tag="w")
                        for dc in range(DC):
                            nc.sync.dma_start(WV[:, dc, :], wvT_r[:, dc, :])
                        # LN1-full + K + V, pipelined per token tile
                        with tc.tile_pool(name="xn1p", bufs=1) as xn1p:
                            xn1 = xn1p.tile([128, DC, S], BF16, tag="xn1")
                            for ti in range(NT):
                                tsl = slice(ti * TS, (ti + 1) * TS)
                                for dc in range(DC):
                                    nc.sync.dma_start(xn1[:, dc, tsl],
                                                      xTbf_r[:, dc, tsl])
                            for ti in range(NT):
                                sl = slice(ti * TS, (ti + 1) * TS)
                                ln_tile(xn1, xn1, ti, TS, lnpsA, bcpsA, 0, 1)
                                for ec in range(DC):
                                    ps = pps.tile([128, TS], FP32, tag="pps")
                                    for dc in range(DC):
                                        nc.tensor.matmul(
                                            ps,
                                            WK[:, dc,
                                               ec * 128:(ec + 1) * 128],
                                            xn1[:, dc, sl],
                                            start=(dc == 0),
                                            stop=(dc == DC - 1))
                                    nc.scalar.activation(KT[:, ec, sl], ps,
                                                         ACT.Copy)
                                EH = min(512, D)
                                for tcn in range(ti * 4, ti * 4 + 4):
                                    for eh in range(D // EH):
                                        esl = slice(eh * EH, (eh + 1) * EH)
                                        ps = pps.tile([128, EH], FP32,
                                                      tag="pps")
                                        for dc in range(DC):
                                            nc.tensor.matmul(
                                                ps,
                                                xn1[:, dc,
                                                    tcn * 128:
                                                    (tcn + 1) * 128],
                                                WV[:, dc, esl],
                                                start=(dc == 0),
                                                stop=(dc == DC - 1))
                                        nc.scalar.activation(
                                            VT[:, tcn, esl], ps, ACT.Copy)
                        # LN1-own + Q, per token tile
                        WQ = wp.tile([128, DC, D], BF16, tag="w")
                        for dc in range(DC):
                            nc.sync.dma_start(WQ[:, dc, :], wqT_r[:, dc, :])
                        with tc.tile_pool(name="xn1op", bufs=1) as xn1op:
                            xn1o = xn1op.tile([128, DC, OWN], BF16,
                                              tag="xn1o")
                            for tj in range(NTO):
                                tsl = slice(tj * TSO, (tj + 1) * TSO)
                                for dc in range(DC):
                                    nc.sync.dma_start(xn1o[:, dc, tsl],
                                                      xTobf_r[:, dc, tsl])
                            for tj in range(NTO):
                                sl = slice(tj * TSO, (tj + 1) * TSO)
                                ln_tile(xn1o, xn1o, tj, TSO, lnpsA, bcpsA,
                                        0, 1)
                                for ec in range(DC):
                                    ps = pps.tile([128, TSO], FP32,
                                                  tag="pps")
                                    for dc in range(DC):
                                        nc.tensor.matmul(
                                            ps,
                                            WQ[:, dc,
                                               ec * 128:(ec + 1) * 128],
                                            xn1o[:, dc, sl],
                                            start=(dc == 0),
                                            stop=(dc == DC - 1))
                                    nc.scalar.activation(
                                        QT[:, ec, sl], ps, ACT.Copy,
                                        scale=scale_q)

                    # ---- attention + folded LN2 ----
                    with (
                        tc.tile_pool(name="s_ps", bufs=2,
                                     space="PSUM") as sps,
                        tc.tile_pool(name="av_ps", bufs=2,
                                     space="PSUM") as avp,
                        tc.tile_pool(name="dn_ps", bufs=1,
                                     space="PSUM") as dnp,
                        tc.tile_pool(name="bc1_ps", bufs=1,
                                     space="PSUM") as bc1,
                        tc.tile_pool(name="ln2_ps", bufs=1,
                                     space="PSUM") as ln2ps,
                        tc.tile_pool(name="wop", bufs=1) as wop,
                        tc.tile_pool(name="ptp", bufs=16) as ptp,
                        tc.tile_pool(name="pep", bufs=3) as pep,
                        tc.tile_pool(name="attw", bufs=2) as attw,
                        tc.tile_pool(name="denr", bufs=2) as denr,
                    ):
                        WO = wop.tile([128, DC, D], BF16, tag="WO")
                        for dc in range(DC):
                            nc.sync.dma_start(WO[:, dc, :], woT_r[:, dc, :])
                        for p in range(P):
                            qsl = slice(p * 256, (p + 1) * 256)
                            nkt = (p + 1) * 4
                            dn = dnp.tile([1, 256], FP32, tag="dn")
                            pts = []
                            for kt in range(nkt):
                                s_ps = sps.tile([128, 256], FP32, tag="s")
                                for ec in range(DC):
                                    nc.tensor.matmul(
                                        s_ps,
                                        KT[:, ec, kt * 128:(kt + 1) * 128],
                                        QT[:, ec, qsl],
                                        start=(ec == 0), stop=(ec == DC - 1))
                                pt = ptp.tile([128, 256], BF16, tag="pt")
                                if kt // 4 == p:
                                    pe = pep.tile([128, 256], BF16, tag="pe")
                                    nc.scalar.activation(pe, s_ps, ACT.Exp)
                                    nc.vector.tensor_mul(
                                        pt, pe, maskt[:, kt % 4, :])
                                else:
                                    nc.scalar.activation(pt, s_ps, ACT.Exp)
                                nc.tensor.matmul(dn, ones_col, pt,
                                                 start=(kt == 0),
                                                 stop=(kt == nkt - 1))
                                pts.append(pt)
                            den = denr.tile([1, 256], F32R, tag="den")
                            nc.vector.tensor_copy(den, dn)
                            with nc.allow_low_precision(
                                    reason="softmax denom"):
                                nc.vector.reciprocal(den, den)
                            den_b = bc1.tile([128, 256], FP32, tag="bc1")
                            nc.tensor.matmul(den_b, ones_row, den)
                            den_sb = attw.tile([128, 256], FP32, tag="densb",
                                               bufs=1)
                            nc.scalar.activation(den_sb, den_b, ACT.Copy)
                            ctxn = attw.tile([128, DC, 256], BF16,
                                             tag="ctxn")
                            for dc in range(DC):
                                cps = avp.tile([128, 256], FP32, tag="av")
                                for kt in range(nkt):
                                    nc.tensor.matmul(
                                        cps,
                                        VT[:, kt, dc * 128:(dc + 1) * 128],
                                        pts[kt], start=(kt == 0),
                                        stop=(kt == nkt - 1))
                                nc.vector.tensor_mul(ctxn[:, dc, :], cps,
                                                     den_sb)
                            # O-projection + residual + LN2 stats
                            xo_t = attw.tile([128, DC, 256], FP32,
                                             tag="xo_t", bufs=1)
                            y1_t = attw.tile([128, DC, 256], FP32,
                                             tag="y1_t", bufs=1)
                            ps2_sx = ln2ps.tile([1, 256], FP32, tag="ps2sx")
                            ps2_sq = ln2ps.tile([1, 256], FP32, tag="ps2sq")
                            for ec in range(DC):
                                nc.sync.dma_start(xo_t[:, ec, :],
                                                  xTown_r[:, ec, qsl])
                                ops_t = avp.tile([128, 256], FP32, tag="av")
                                for dc in range(DC):
                                    nc.tensor.matmul(
                                        ops_t,
                                        WO[:, dc, ec * 128:(ec + 1) * 128],
                                        ctxn[:, dc, :],
                                        start=(dc == 0), stop=(dc == DC - 1))
                                nc.vector.tensor_add(y1_t[:, ec, :], ops_t,
                                                     xo_t[:, ec, :])
                                nc.sync.dma_start(y1d_r[:, ec, qsl],
                                                  y1_t[:, ec, :])
                                ybf = work.tile([128, 256], BF16, tag="ybf")
                                nc.scalar.activation(ybf, y1_t[:, ec, :],
                                                     ACT.Copy)
                                sq = work.tile([128, 256], BF16, tag="lnsq")
                                nc.scalar.square(sq, ybf)
                                nc.tensor.matmul(ps2_sx, ones_col, ybf,
                                                 start=(ec == 0),
                                                 stop=(ec == DC - 1))
                                nc.tensor.matmul(ps2_sq, ones_col, sq,
                                                 start=(ec == 0),
                                                 stop=(ec == DC - 1))
                            # LN2 rows + apply for this pair's columns
                            mu2 = work.tile([1, 256], F32R, tag="r_mu")
                            rs2 = work.tile([1, 256], F32R, tag="r_rs")
                            msq2 = work.tile([1, 256], F32R, tag="r_msq")
                            nc.scalar.activation(mu2, ps2_sx, ACT.Copy,
                                                 scale=1.0 / D)
                            nc.scalar.activation(rs2, ps2_sq, ACT.Copy,
                                                 scale=1.0 / D)
                            nc.vector.tensor_mul(msq2, mu2, mu2)
                            nc.vector.tensor_sub(rs2, rs2, msq2)
                            nc.scalar.activation(rs2, rs2, ACT.Sqrt,
                                                 bias=eps_t)
                            with nc.allow_low_precision(reason="rstd f32r"):
                                nc.vector.reciprocal(rs2, rs2)
                            mb2 = bc1.tile([128, 256], FP32, tag="bc1")
                            nc.tensor.matmul(mb2, ones_row, mu2)
                            mb2_sb = work.tile([128, 256], FP32,
                                               tag="mb_sb2")
                            nc.scalar.activation(mb2_sb, mb2, ACT.Copy)
                            rb2 = bc1.tile([128, 256], FP32, tag="bc1")
                            nc.tensor.matmul(rb2, ones_row, rs2)
                            rb2_sb = work.tile([128, 256], FP32,
                                               tag="rb_sb2")
                            nc.scalar.activation(rb2_sb, rb2, ACT.Copy)
                            for dc in range(DC):
                                t1 = work.tile([128, 256], FP32, tag="lnt2")
                                nc.vector.tensor_sub(t1, y1_t[:, dc, :],
                                                     mb2_sb)
                                if apply_ln_affine:
                                    nc.vector.scalar_tensor_tensor(
                                        xn2[:, dc, qsl], t1,
                                        lnp_t[:, 2, dc:dc + 1], rb2_sb,
                                        AOP.mult, AOP.mult)
                                    nc.vector.tensor_scalar_add(
                                        xn2[:, dc, qsl], xn2[:, dc, qsl],
                                        lnp_t[:, 3, dc:dc + 1])
                                else:
                                    nc.vector.tensor_mul(xn2[:, dc, qsl],
                                                         t1, rb2_sb)

                # ---- FFN ----
                with tc.tile_pool(name="hpool", bufs=1) as hp:
                    h = hp.tile([128, FC, OWN], BF16, tag="h")
                    with (
                        tc.tile_pool(name="bigw", bufs=2) as bw,
                        tc.tile_pool(name="ffn_ps", bufs=3,
                                     space="PSUM") as fps,
                        tc.tile_pool(name="outp", bufs=3) as otp,
                        tc.tile_pool(name="ffap", bufs=1) as fap,
                    ):
                        FQ = FC // 4
                        for quar in range(4):
                            w1 = bw.tile([128, DC, FQ * 128], BF16,
                                         tag="bigw")
                            for dc in range(DC):
                                nc.sync.dma_start(
                                    w1[:, dc, :],
                                    fc1T_r[:, dc, quar * FQ * 128:
                                           (quar + 1) * FQ * 128])
                            for fi in range(FQ):
                                fc = quar * FQ + fi
                                for tj in range(NTO):
                                    sl = slice(tj * TSO, (tj + 1) * TSO)
                                    ps = fps.tile([128, TSO], FP32,
                                                  tag="fps")
                                    for dc in range(DC):
                                        nc.tensor.matmul(
                                            ps,
                                            w1[:, dc,
                                               fi * 128:(fi + 1) * 128],
                                            xn2[:, dc, sl],
                                            start=(dc == 0),
                                            stop=(dc == DC - 1))
                                    nc.scalar.activation(
                                        h[:, fc, sl], ps, ACT.Relu,
                                        bias=fc1b_t[:, fc:fc + 1])
                        # fc2 in quarters accumulated via ffa
                        ffa = fap.tile([128, DC, OWN], FP32, tag="ffa")
                        for quar in range(4):
                            w2 = bw.tile([128, FQ, D], BF16, tag="bigw")
                            for fi in range(FQ):
                                nc.sync.dma_start(
                                    w2[:, fi, :],
                                    fc2T_r[:, quar * FQ + fi, :])
                            for dc in range(DC):
                                for tj in range(NTO):
                                    sl = slice(tj * TSO, (tj + 1) * TSO)
                                    ps = fps.tile([128, TSO], FP32,
                                                  tag="fps")
                                    for fi in range(FQ):
                                        fc = quar * FQ + fi
                                        nc.tensor.matmul(
                                            ps,
                                            w2[:, fi,
                                               dc * 128:(dc + 1) * 128],
                                            h[:, fc, sl],
                                            start=(fi == 0),
                                            stop=(fi == FQ - 1))
                                    if quar == 0:
                                        nc.vector.tensor_scalar_add(
                                            ffa[:, dc, sl], ps,
                                            fc2b_t[:, dc:dc + 1])
                                    elif quar < 3:
                                        nc.vector.tensor_add(
                                            ffa[:, dc, sl], ffa[:, dc, sl],
                                            ps)
                                    else:
                                        yt = otp.tile([128, TSO], FP32,
                                                      tag="yt2")
                                        nc.sync.dma_start(yt,
                                                          y1d_r[:, dc, sl])
                                        ot = otp.tile([128, TSO], FP32,
                                                      tag="ot")
                                        nc.vector.tensor_add(
                                            ot, ps, ffa[:, dc, sl])
                                        nc.vector.tensor_add(ot, ot, yt)
                                        nc.sync.dma_start(outT_r[:, dc, sl],
                                                          ot)

    if surgery:
        _split_drain_waits(nc)
    return nc


# ---------------- host side ----------------

_NC_CACHE = {}


def _get_nc(S, D, F, apply_ln_affine, repeat=1):
    key = (S, D, F, apply_ln_affine, repeat)
    if key not in _NC_CACHE:
        _NC_CACHE[key] = build_decoder_nc(S, D, F, apply_ln_affine,
                                          repeat=repeat)
    return _NC_CACHE[key]


def make_in_maps(x, W_q, W_k, W_v, W_o, fc1_w, fc1_b, fc2_w, fc2_b,
                 ln1_g, ln1_b, ln2_g, ln2_b, apply_ln_affine):
    B, S, D = x.shape
    bf = ml_dtypes.bfloat16
    shared = {
        "wqT": np.ascontiguousarray(W_q.T).astype(bf),
        "wkT": np.ascontiguousarray(W_k.T).astype(bf),
        "wvT": np.ascontiguousarray(W_v.T).astype(bf),
        "woT": np.ascontiguousarray(W_o.T).astype(bf),
        "fc1T": np.ascontiguousarray(fc1_w.T).astype(bf),
        "fc2T": np.ascontiguousarray(fc2_w.T).astype(bf),
        "fc1b": np.ascontiguousarray(fc1_b, dtype=np.float32),
        "fc2b": np.ascontiguousarray(fc2_b, dtype=np.float32),
    }
    if apply_ln_affine:
        shared["lnp"] = np.ascontiguousarray(
            np.stack([ln1_g, ln1_b, ln2_g, ln2_b]), dtype=np.float32)
    in_maps, stripes = [], []
    for c in range(2 * B):
        b, hh = c // 2, c % 2
        stripe = (np.arange(S) % 512) // 256 == hh
        stripes.append((b, stripe))
        xTb = np.ascontiguousarray(x[b].T, dtype=np.float32)
        m = np.zeros((512, 256), dtype=bf)
        tk = np.arange(512)[:, None]
        j = np.arange(256)[None, :]
        m[tk <= j + 256 * hh] = 1.0
        xTo = np.ascontiguousarray(xTb[:, stripe])
        in_maps.append(dict(shared,
                            xTbf=xTb.astype(bf),
                            xTobf=xTo.astype(bf),
                            xTown=xTo,
                            mask01=m))
    return in_maps, stripes


def run_decoder(x, W_q, W_k, W_v, W_o, fc1_w, fc1_b, fc2_w, fc2_b,
                ln1_g, ln1_b, ln2_g, ln2_b, trace=False):
    x = np.asarray(x, dtype=np.float32)
    B, S, D = x.shape
    F = fc1_w.shape[0]
    apply_ln_affine = not (
        np.all(np.asarray(ln1_g) == 1.0) and np.all(np.asarray(ln1_b) == 0.0)
        and np.all(np.asarray(ln2_g) == 1.0)
        and np.all(np.asarray(ln2_b) == 0.0))
    nc = _get_nc(S, D, F, apply_ln_affine)
    in_maps, stripes = make_in_maps(
        x, np.asarray(W_q), np.asarray(W_k), np.asarray(W_v),
        np.asarray(W_o), np.asarray(fc1_w), np.asarray(fc1_b),
        np.asarray(fc2_w), np.asarray(fc2_b), np.asarray(ln1_g),
        np.asarray(ln1_b), np.asarray(ln2_g), np.asarray(ln2_b),
        apply_ln_affine)
    res = run_bass_kernel_spmd(nc, in_maps, core_ids=list(range(2 * B)),
                               trace=trace)
    out = np.empty((B, S, D), dtype=np.float32)
    for c in range(2 * B):
        b, stripe = stripes[c]
        out[b, stripe, :] = res.results[c]["outT"].T
    return out, res


def kernel(**inputs):
    out, _ = run_decoder(**inputs)
    return out


def _build_pjrt_fn(nc, in_maps):
    """Build a non-donating jitted executor + device-resident args."""
    import jax
    from jax.sharding import Mesh, PartitionSpec, NamedSharding
    from jax.experimental.shard_map import shard_map
    from concourse import bass2jax

    n_cores = len(in_maps)
    bass2jax.install_neuronx_cc_hook()
    partition_name = (nc.partition_id_tensor.name
                      if nc.partition_id_tensor else None)
    in_names, out_names, out_avals, zero_outs = [], [], [], []
    for alloc in nc.m.functions[0].allocations:
        if not isinstance(alloc, mybir.MemoryLocationSet):
            continue
        name = alloc.memorylocations[0].name
        if alloc.kind == "ExternalInput":
            if name != partition_name:
                in_names.append(name)
        elif alloc.kind == "ExternalOutput":
            shape = tuple(alloc.tensor_shape)
            dtype = mybir.dt.np(alloc.dtype)
            out_names.append(name)
            out_avals.append(jax.core.ShapedArray(shape, dtype))
            zero_outs.append(np.zeros(shape, dtype))
    n_params = len(in_names)
    in_names.extend(out_names)
    if partition_name is not None:
        in_names.append(partition_name)

    def _body(*args):
        operands = list(args)
        if partition_name is not None:
            operands.append(bass2jax.partition_id_tensor())
        return tuple(bass2jax._bass_exec_p.bind(
            *operands, out_avals=tuple(out_avals), in_names=tuple(in_names),
            out_names=tuple(out_names), lowering_input_output_aliases=(),
            sim_require_finite=True, sim_require_nnan=True, nc=nc))

    devices = jax.devices()[:n_cores]
    mesh = Mesh(np.asarray(devices), ("core",))
    fn = jax.jit(shard_map(
        _body, mesh=mesh,
        in_specs=(PartitionSpec("core"),) * (n_params + len(out_names)),
        out_specs=(PartitionSpec("core"),) * len(out_names),
        check_rep=False), keep_unused=True)
    sh = NamedSharding(mesh, PartitionSpec("core"))
    args = []
    for i in range(n_params):
        cat = np.concatenate([np.asarray(in_maps[c][in_names[i]])
                              for c in range(n_cores)], axis=0)
        args.append(jax.device_put(cat, sh))
    for z in zero_outs:
        args.append(jax.device_put(
            np.zeros((n_cores * z.shape[0], *z.shape[1:]), z.dtype), sh))
    return fn, args


def measure_body_ns(iters=3, n1=8, n2=40, **inputs):
    """Isolate per-execution NEFF body time from dispatch overhead: slope of
    async-pipelined executions, differenced between repeat=1 and repeat=4
    NEFFs.  Returns (body_ns, slope1_ns)."""
    import time
    import jax

    x = np.asarray(inputs["x"], dtype=np.float32)
    B, S, D = x.shape
    F = np.asarray(inputs["fc1_w"]).shape[0]
    in_maps, _ = make_in_maps(
        x, *[np.asarray(inputs[k]) for k in
             ("W_q", "W_k", "W_v", "W_o", "fc1_w", "fc1_b", "fc2_w", "fc2_b",
              "ln1_g", "ln1_b", "ln2_g", "ln2_b")], False)

    def slope(repeat):
        nc = _get_nc(S, D, F, False, repeat=repeat)
        fn, args = _build_pjrt_fn(nc, in_maps)
        o = fn(*args)
        jax.block_until_ready(o)
        ts = {}
        for N in (n1, n2):
            best = float("inf")
            for _ in range(iters):
                t0 = time.perf_counter()
                for _i in range(N):
                    o = fn(*args)
                jax.block_until_ready(o)
                best = min(best, time.perf_counter() - t0)
            ts[N] = best
        return (ts[n2] - ts[n1]) / (n2 - n1)

    s1 = slope(1)
    s4 = slope(4)
    return (s4 - s1) / 3 * 1e9, s1 * 1e9
